# revision 18
# baseline (speedup 1.0000x reference)
"""Trainium2 Bass kernel for MoGNN forward (global mean-pool + linear).

The model's conv outputs are discarded; the result depends only on x:
    pooled[g] = mean over nodes n with batch[n] == g of x[n]   # [1024, 512]
    out = pooled @ W.T + b                                     # [1024, 7]

batch ids are sorted, so nodes of each graph are contiguous. We shard by
GRAPHS: core k owns graphs [128k, 128k+128) and exactly the contiguous row
range of x belonging to them (padded to a tile multiple). No collectives.

Per 128-node tile, on device:
  - DVE builds an exact one-hot matrix oh[n, g] = (batch_local[n] == g)
    with a single tensor_scalar (is_equal vs an iota row).
  - PE matmul (fp16 in, fp32 PSUM accumulate, full rate at N=512) does
    psum[128 graphs, 512 feats] += oh.T @ x_tile.
Epilogue: PSUM -> SBUF with a per-graph 1/count scale (mean pool), 4x PE
transpose to feat-major, 4 fp32 matmuls against the pre-transposed
classifier W, bias added as a per-partition scalar; each core writes
out[7, 128] and the host concatenates/transposes to [1024, 7].

x is shipped as fp16 (11-bit effective mantissa; accumulation stays fp32 in
PSUM) — measured end-to-end relative error vs the fp32 reference ~2e-4,
comparable to the fp32r (tf32-like) matmul path while halving HBM traffic.
"""

import numpy as np

NCORES = 8
G = 1024            # total graphs
GPC = G // NCORES   # graphs per core = 128
F = 512             # feature dim
P = 128             # partition / node-tile size
CHUNK = 8           # node tiles per DMA chunk (1 MB fp16 transfers)

_compiled_cache = {}


def _build(ntiles):
    """Build + compile the per-core Bass kernel for a shard of `ntiles` node tiles."""
    from concourse import bacc, tile, mybir

    f32 = mybir.dt.float32
    f16 = mybir.dt.float16
    eq = mybir.AluOpType.is_equal
    mult = mybir.AluOpType.mult
    add = mybir.AluOpType.add

    nrows = ntiles * P
    # chunk boundaries: small leading chunks so the PE pipeline starts early,
    # then CHUNK-tile steady state (last chunk may be short)
    chunks = []
    t0 = 0
    for ramp in (2, 6):
        if t0 < ntiles:
            clen = min(ramp, ntiles - t0)
            chunks.append((t0, clen))
            t0 += clen
    while t0 < ntiles:
        chunks.append((t0, min(CHUNK, ntiles - t0)))
        t0 += CHUNK

    nc = bacc.Bacc(
        "TRN2",
        target_bir_lowering=False,
        debug=False,
        num_devices=NCORES,
    )

    # x shard in partition-major layout: xs[p, t*F + m] = x[t*128 + p, m],
    # so each chunk is one contiguous multi-KB run per partition
    x_d = nc.dram_tensor("xs", [P, (nrows // P) * F], f16, kind="ExternalInput")
    bl_d = nc.dram_tensor("bl", [P, ntiles], f16, kind="ExternalInput")
    icnt_d = nc.dram_tensor("icnt", [GPC, 1], f32, kind="ExternalInput")
    iota_d = nc.dram_tensor("iota", [P, GPC], f16, kind="ExternalInput")
    ident_d = nc.dram_tensor("ident", [P, P], f32, kind="ExternalInput")
    wtr_d = nc.dram_tensor("wtr", [P, 4 * 7], f32, kind="ExternalInput")
    bv_d = nc.dram_tensor("bvec", [7, 1], f32, kind="ExternalInput")
    out_d = nc.dram_tensor("out", [7, GPC], f32, kind="ExternalOutput")

    with tile.TileContext(nc) as tc:
        with (
            tc.tile_pool(name="const", bufs=1) as constp,
            tc.tile_pool(name="xin", bufs=4) as xp,
            tc.tile_pool(name="oh", bufs=4) as ohp,
            tc.tile_pool(name="acc", bufs=1, space="PSUM") as accp,
            tc.tile_pool(name="tps", bufs=2, space="PSUM") as tpsp,
            tc.tile_pool(name="outp", bufs=1, space="PSUM") as outpp,
            tc.tile_pool(name="sb", bufs=2) as sbp,
        ):
            # constants go on the scalar-engine HWDGE ring so they don't
            # delay the x-chunk stream on the sync ring
            iota_t = constp.tile([P, GPC], f16)
            nc.scalar.dma_start(iota_t[:], iota_d.ap())
            bl_t = constp.tile([P, ntiles], f16)
            nc.scalar.dma_start(bl_t[:], bl_d.ap())
            ident_t = constp.tile([P, P], f32)
            nc.scalar.dma_start(ident_t[:], ident_d.ap())
            wtr_t = constp.tile([P, 4 * 7], f32)
            nc.scalar.dma_start(wtr_t[:], wtr_d.ap())
            bv_t = constp.tile([7, 1], f32)
            nc.scalar.dma_start(bv_t[:], bv_d.ap())
            icnt_t = constp.tile([GPC, 1], f32)
            nc.scalar.dma_start(icnt_t[:], icnt_d.ap())

            acc = accp.tile([GPC, F], f32)
            xr = x_d.ap().rearrange("p (t m) -> p t m", m=F)

            iota_rep = iota_t[:].rearrange("p (a g) -> p a g", a=1)
            t = 0
            for ci, (c0, clen) in enumerate(chunks):
                xt = xp.tile([P, CHUNK, F], f16, tag="xt")
                dma_eng = nc.sync if ci % 2 == 0 else nc.scalar
                dma_eng.dma_start(
                    xt[:, :clen, :],
                    xr[:, c0 : c0 + clen, :],
                )
                # one-hot for the whole chunk in one DVE op via broadcast APs:
                # oh[p, n, g] = (iota[g] == bl[p, c0+n])
                oh = ohp.tile([P, CHUNK, GPC], f16, tag="oh")
                nc.vector.tensor_tensor(
                    oh[:, :clen, :],
                    iota_rep.broadcast_to([P, clen, GPC]),
                    bl_t[:, c0 : c0 + clen]
                    .rearrange("p (n a) -> p n a", a=1)
                    .broadcast_to([P, clen, GPC]),
                    op=eq,
                )
                for n in range(clen):
                    nc.tensor.matmul(
                        acc[:],
                        oh[:, n, :],
                        xt[:, n, :],
                        start=(t == 0),
                        stop=(t == ntiles - 1),
                    )
                    t += 1

            # pooled [graphs, feats] = acc * (1/count[g])  -> SBUF
            pooled = sbp.tile([GPC, F], f32)
            nc.vector.tensor_scalar(
                pooled[:], acc[:], icnt_t[:, 0:1], None, op0=mult
            )

            # transpose to feat-major, then classifier matmuls
            ptall = sbp.tile([P, 4, P], f32)
            for j in range(4):
                tp = tpsp.tile([P, P], f32)
                nc.tensor.transpose(tp[:], pooled[:, j * P : (j + 1) * P], ident_t[:])
                nc.vector.tensor_copy(ptall[:, j, :], tp[:])

            out_ps = outpp.tile([7, GPC], f32)
            for j in range(4):
                nc.tensor.matmul(
                    out_ps[:],
                    wtr_t[:, j * 7 : (j + 1) * 7],
                    ptall[:, j, :],
                    start=(j == 0),
                    stop=(j == 3),
                )

            out_sb = sbp.tile([7, GPC], f32)
            nc.vector.tensor_scalar(
                out_sb[:], out_ps[:], bv_t[:, 0:1], None, op0=add
            )
            nc.sync.dma_start(out_d.ap(), out_sb[:])

    nc.compile()
    return nc


def _get_compiled(ntiles):
    if ntiles not in _compiled_cache:
        _compiled_cache[ntiles] = _build(ntiles)
    return _compiled_cache[ntiles]


def _prep_in_maps(x16, batch, W, b, ntiles, bounds, inv_counts):
    cap = ntiles * P
    iota = np.tile(np.arange(GPC, dtype=np.float16)[None, :], (P, 1))
    ident = np.eye(P, dtype=np.float32)
    # wtr[p, c*7 + j] = W.T[c*128 + p, j] = W[j, c*128 + p]
    wtr = np.ascontiguousarray(
        W.T.reshape(4, P, 7).transpose(1, 0, 2).reshape(P, 28)
    ).astype(np.float32)
    bv = b.reshape(7, 1).astype(np.float32)

    in_maps = []
    for k in range(NCORES):
        lo, hi = int(bounds[k]), int(bounds[k + 1])
        n = hi - lo
        xs = np.zeros((cap, F), dtype=np.float16)
        xs[:n] = x16[lo:hi]
        # partition-major: xs_t[p, t, m] = xs[t*128 + p, m]
        xs = np.ascontiguousarray(
            xs.reshape(ntiles, P, F).transpose(1, 0, 2)
        ).reshape(P, ntiles * F)
        blv = np.full((cap,), -1.0, dtype=np.float16)
        blv[:n] = (batch[lo:hi] - GPC * k).astype(np.float16)
        in_maps.append(
            {
                "xs": xs,
                "bl": np.ascontiguousarray(blv.reshape(ntiles, P).T),
                "icnt": inv_counts[GPC * k : GPC * (k + 1)].reshape(GPC, 1),
                "iota": iota,
                "ident": ident,
                "wtr": wtr,
                "bvec": bv,
            }
        )
    return in_maps


_last_result = None  # test harness can read exec_time_ns / trace from here


def kernel(x, edge_index, edge_attr, batch_size, W, b):
    from concourse import bass_utils

    global _last_result

    x16 = np.asarray(x, dtype=np.float16)
    batch = np.asarray(batch_size).astype(np.int64)
    W = np.asarray(W, dtype=np.float32)
    b = np.asarray(b, dtype=np.float32)

    counts = np.bincount(batch, minlength=G)
    inv_counts = (1.0 / np.maximum(counts, 1)).astype(np.float32)
    bounds = np.searchsorted(batch, np.arange(0, G + 1, GPC))
    max_rows = int(np.diff(bounds).max())
    ntiles = max(-(-max_rows // P), 1)

    nc = _get_compiled(ntiles)
    in_maps = _prep_in_maps(x16, batch, W, b, ntiles, bounds, inv_counts)

    res = bass_utils.run_bass_kernel_spmd(
        nc, in_maps, core_ids=list(range(NCORES))
    )
    _last_result = res

    # each core returns out [7, 128] for its graphs; assemble [1024, 7]
    out = np.concatenate(
        [np.asarray(res.results[k]["out"]).T for k in range(NCORES)], axis=0
    )
    return np.ascontiguousarray(out.astype(np.float32))


# revision 24
# speedup vs baseline: 1.2383x; 1.2383x over previous
"""Trainium2 Bass kernel for MoGNN forward (global mean-pool + linear).

The model's conv outputs are discarded; the result depends only on x:
    pooled[g] = mean over nodes n with batch[n] == g of x[n]   # [1024, 512]
    out = pooled @ W.T + b                                     # [1024, 7]

batch ids are sorted, so nodes of each graph are contiguous. We shard by
GRAPHS: core k owns graphs [128k, 128k+128) and exactly the contiguous row
range of x belonging to them (padded to a tile multiple). No collectives.

Per 128-node tile, on device:
  - DVE builds an exact one-hot matrix oh[n, g] = (batch_local[n] == g)
    with a single tensor_scalar (is_equal vs an iota row).
  - PE matmul (fp16 in, fp32 PSUM accumulate, full rate at N=512) does
    psum[128 graphs, 512 feats] += oh.T @ x_tile.
Epilogue: PSUM -> SBUF with a per-graph 1/count scale (mean pool), 4x PE
transpose to feat-major, 4 fp32 matmuls against the pre-transposed
classifier W, bias added as a per-partition scalar; each core writes
out[7, 128] and the host concatenates/transposes to [1024, 7].

x is shipped as fp16 (11-bit effective mantissa; accumulation stays fp32 in
PSUM) — measured end-to-end relative error vs the fp32 reference ~2e-4,
comparable to the fp32r (tf32-like) matmul path while halving HBM traffic.
"""

import numpy as np

NCORES = 8
G = 1024            # total graphs
GPC = G // NCORES   # graphs per core = 128
F = 512             # feature dim
P = 128             # partition / node-tile size
CHUNK = 8           # node tiles per DMA chunk (1 MB fp16 transfers)

_compiled_cache = {}


def _chunk_plan(ntiles):
    """Chunk boundaries: small leading chunks so the PE pipeline starts early,
    then CHUNK-tile steady state (last chunk may be short)."""
    chunks = []
    t0 = 0
    for ramp in (2, 6):
        if t0 < ntiles:
            clen = min(ramp, ntiles - t0)
            chunks.append((t0, clen))
            t0 += clen
    while t0 < ntiles:
        chunks.append((t0, min(CHUNK, ntiles - t0)))
        t0 += CHUNK
    return chunks


def _build(ntiles):
    """Build + compile the per-core Bass kernel for a shard of `ntiles` node tiles."""
    from concourse import bacc, tile, mybir

    f32 = mybir.dt.float32
    f16 = mybir.dt.float16
    eq = mybir.AluOpType.is_equal
    mult = mybir.AluOpType.mult
    add = mybir.AluOpType.add

    nrows = ntiles * P
    chunks = _chunk_plan(ntiles)

    nc = bacc.Bacc(
        "TRN2",
        target_bir_lowering=False,
        debug=False,
        num_devices=NCORES,
    )

    # x shard laid out chunk-contiguous and partition-major inside each chunk:
    # for chunk (c0, clen), the DRAM block holds block[p, t, m] = x[(c0+t)*128+p, m]
    # so the whole chunk is one contiguous region and each partition reads one
    # contiguous multi-KB run
    x_d = nc.dram_tensor("xs", [nrows * F], f16, kind="ExternalInput")
    bl_d = nc.dram_tensor("bl", [P, ntiles], f16, kind="ExternalInput")
    icnt_d = nc.dram_tensor("icnt", [GPC, 1], f32, kind="ExternalInput")
    iota_d = nc.dram_tensor("iota", [P, GPC], f16, kind="ExternalInput")
    ident_d = nc.dram_tensor("ident", [P, P], f32, kind="ExternalInput")
    wtr_d = nc.dram_tensor("wtr", [P, 4 * 7], f32, kind="ExternalInput")
    bv_d = nc.dram_tensor("bvec", [7, 1], f32, kind="ExternalInput")
    out_d = nc.dram_tensor("out", [7, GPC], f32, kind="ExternalOutput")

    with tile.TileContext(nc) as tc:
        with (
            tc.tile_pool(name="const", bufs=1) as constp,
            tc.tile_pool(name="xin", bufs=4) as xp,
            tc.tile_pool(name="oh", bufs=4) as ohp,
            tc.tile_pool(name="acc", bufs=1, space="PSUM") as accp,
            tc.tile_pool(name="tps", bufs=2, space="PSUM") as tpsp,
            tc.tile_pool(name="outp", bufs=1, space="PSUM") as outpp,
            tc.tile_pool(name="sb", bufs=2) as sbp,
        ):
            # constants go on the scalar-engine HWDGE ring so they don't
            # delay the x-chunk stream on the sync ring
            iota_t = constp.tile([P, GPC], f16)
            nc.scalar.dma_start(iota_t[:], iota_d.ap())
            bl_t = constp.tile([P, ntiles], f16)
            nc.scalar.dma_start(bl_t[:], bl_d.ap())
            ident_t = constp.tile([P, P], f32)
            nc.scalar.dma_start(ident_t[:], ident_d.ap())
            wtr_t = constp.tile([P, 4 * 7], f32)
            nc.scalar.dma_start(wtr_t[:], wtr_d.ap())
            bv_t = constp.tile([7, 1], f32)
            nc.scalar.dma_start(bv_t[:], bv_d.ap())
            icnt_t = constp.tile([GPC, 1], f32)
            nc.scalar.dma_start(icnt_t[:], icnt_d.ap())

            acc = accp.tile([GPC, F], f32)
            x_flat = x_d.ap()

            iota_rep = iota_t[:].rearrange("p (a g) -> p a g", a=1)
            t = 0
            for c0, clen in chunks:
                xt = xp.tile([P, CHUNK, F], f16, tag="xt")
                chunk_ap = x_flat[c0 * P * F : (c0 + clen) * P * F].rearrange(
                    "(p t m) -> p t m", p=P, m=F
                )
                nc.sync.dma_start(xt[:, :clen, :], chunk_ap)
                # one-hot for the whole chunk in one DVE op via broadcast APs:
                # oh[p, n, g] = (iota[g] == bl[p, c0+n])
                oh = ohp.tile([P, CHUNK, GPC], f16, tag="oh")
                nc.vector.tensor_tensor(
                    oh[:, :clen, :],
                    iota_rep.broadcast_to([P, clen, GPC]),
                    bl_t[:, c0 : c0 + clen]
                    .rearrange("p (n a) -> p n a", a=1)
                    .broadcast_to([P, clen, GPC]),
                    op=eq,
                )
                for n in range(clen):
                    nc.tensor.matmul(
                        acc[:],
                        oh[:, n, :],
                        xt[:, n, :],
                        start=(t == 0),
                        stop=(t == ntiles - 1),
                    )
                    t += 1

            # pooled [graphs, feats] = acc * (1/count[g])  -> SBUF
            pooled = sbp.tile([GPC, F], f32)
            nc.vector.tensor_scalar(
                pooled[:], acc[:], icnt_t[:, 0:1], None, op0=mult
            )

            # transpose to feat-major, then classifier matmuls
            ptall = sbp.tile([P, 4, P], f32)
            for j in range(4):
                tp = tpsp.tile([P, P], f32)
                nc.tensor.transpose(tp[:], pooled[:, j * P : (j + 1) * P], ident_t[:])
                nc.vector.tensor_copy(ptall[:, j, :], tp[:])

            out_ps = outpp.tile([7, GPC], f32)
            for j in range(4):
                nc.tensor.matmul(
                    out_ps[:],
                    wtr_t[:, j * 7 : (j + 1) * 7],
                    ptall[:, j, :],
                    start=(j == 0),
                    stop=(j == 3),
                )

            out_sb = sbp.tile([7, GPC], f32)
            nc.vector.tensor_scalar(
                out_sb[:], out_ps[:], bv_t[:, 0:1], None, op0=add
            )
            nc.sync.dma_start(out_d.ap(), out_sb[:])

    nc.compile()
    return nc


def _get_compiled(ntiles):
    if ntiles not in _compiled_cache:
        _compiled_cache[ntiles] = _build(ntiles)
    return _compiled_cache[ntiles]


def _prep_in_maps(x16, batch, W, b, ntiles, bounds, inv_counts):
    cap = ntiles * P
    chunk_plan = _chunk_plan(ntiles)
    iota = np.tile(np.arange(GPC, dtype=np.float16)[None, :], (P, 1))
    ident = np.eye(P, dtype=np.float32)
    # wtr[p, c*7 + j] = W.T[c*128 + p, j] = W[j, c*128 + p]
    wtr = np.ascontiguousarray(
        W.T.reshape(4, P, 7).transpose(1, 0, 2).reshape(P, 28)
    ).astype(np.float32)
    bv = b.reshape(7, 1).astype(np.float32)

    in_maps = []
    for k in range(NCORES):
        lo, hi = int(bounds[k]), int(bounds[k + 1])
        n = hi - lo
        xs = np.zeros((cap, F), dtype=np.float16)
        xs[:n] = x16[lo:hi]
        # chunk-contiguous, partition-major within each chunk
        xs = xs.reshape(ntiles, P, F)
        parts = [
            np.ascontiguousarray(xs[c0 : c0 + clen].transpose(1, 0, 2)).reshape(-1)
            for c0, clen in chunk_plan
        ]
        xs = np.concatenate(parts)
        blv = np.full((cap,), -1.0, dtype=np.float16)
        blv[:n] = (batch[lo:hi] - GPC * k).astype(np.float16)
        in_maps.append(
            {
                "xs": xs,
                "bl": np.ascontiguousarray(blv.reshape(ntiles, P).T),
                "icnt": inv_counts[GPC * k : GPC * (k + 1)].reshape(GPC, 1),
                "iota": iota,
                "ident": ident,
                "wtr": wtr,
                "bvec": bv,
            }
        )
    return in_maps


_last_result = None  # test harness can read exec_time_ns / trace from here


def kernel(x, edge_index, edge_attr, batch_size, W, b):
    from concourse import bass_utils

    global _last_result

    x16 = np.asarray(x, dtype=np.float16)
    batch = np.asarray(batch_size).astype(np.int64)
    W = np.asarray(W, dtype=np.float32)
    b = np.asarray(b, dtype=np.float32)

    counts = np.bincount(batch, minlength=G)
    inv_counts = (1.0 / np.maximum(counts, 1)).astype(np.float32)
    bounds = np.searchsorted(batch, np.arange(0, G + 1, GPC))
    max_rows = int(np.diff(bounds).max())
    ntiles = max(-(-max_rows // P), 1)

    nc = _get_compiled(ntiles)
    in_maps = _prep_in_maps(x16, batch, W, b, ntiles, bounds, inv_counts)

    res = bass_utils.run_bass_kernel_spmd(
        nc, in_maps, core_ids=list(range(NCORES))
    )
    _last_result = res

    # each core returns out [7, 128] for its graphs; assemble [1024, 7]
    out = np.concatenate(
        [np.asarray(res.results[k]["out"]).T for k in range(NCORES)], axis=0
    )
    return np.ascontiguousarray(out.astype(np.float32))
